# revision 17
# baseline (speedup 1.0000x reference)
"""DarkChannelPrior kernel for 8 Trainium2 NeuronCores.

Computes: dark = min over 3 channels of images [16,3,1024,1024], then a
15x15 box-average convolution (padding 7) -> [16,1,1024,1024].

Strategy (streaming slab pipeline, pure data parallel, 2 images/core):
  - The host casts the f32 images to fp8 e4m3 and decodes the device's
    bias-encoded fp8 output back to f32 (y = y8 + 0.25).  Total device
    rel err ~5.6e-3 vs the 2e-2 gate.  HBM traffic: 6.29 MB read +
    2.10 MB written per core (vs 25.2 + 8.4 for the all-f32 pipeline).
  - The DMA-engine pool is charged for the LARGER side of each transfer
    (~330 GB/s per core), so a casting fp8->fp16 DMA costs its fp16
    destination bytes; per-DMA overheads (~1 us on the software-DGE
    path) also matter.  The input is therefore split and batched:
    channels 0,1 ride one gpsimd/SWDGE casting DMA per 4-slab group per
    channel (fp8 HBM -> fp16 SBUF; the only path that can cast),
    channel 2 rides the SP HWDGE ring as raw fp8 in 4-slab batches and
    ScalarE upcasts it per slab (ScalarE has slack).
  - VectorE computes the channel min with two fp16 tensor_tensor ops
    (fp16 engages the DVE 2x_1p mode; fp8 would run at half rate) into
    a per-image fp16 dark buffer [128, 8*1024].
  - TensorE computes the 15-tap ROW sums with up to 3 accumulating
    banded fp16 matmuls per 512-col half (band stationary, dark moving;
    two small bands pull the conv halo from the neighbouring slabs'
    dark columns), both halves into one 2-bank PSUM tile -> ScalarE
    copies PSUM->padbuf (fp16) in ONE op per slab with the 1/225 scale
    AND a -0.25/15 bias fused: the bias makes the 15-tap column window
    sum come out as (boxavg - 0.25), centring the output for fp8 ->
    VectorE computes the 15-tap COLUMN sums in one tensor_tensor_scan
    (state = (t1[v] + state) - t1[v-15], fp32 state; the pad columns
    hold -0.25/15 so border windows get the same -0.25 shift) -> the
    finished bias-encoded fp8 rows go out on the SP HWDGE ring.
  - The per-slab stages are STAGGERED (load | matmuls t-1 | psum copy
    t-2 | scan+store t-3): every engine's in-order queue only sees
    instructions whose deps completed ~2 slabs ago, so nothing
    head-of-line blocks and the DMA pool streams continuously.
"""

import numpy as np
import ml_dtypes

import concourse.bacc as bacc
import concourse.bass as bass
import concourse.tile as tile
import concourse.mybir as mybir
from concourse.bass_utils import run_bass_kernel_spmd

F8 = mybir.dt.float8e4
F16 = mybir.dt.float16
F32 = mybir.dt.float32

KS = 15
PAD = KS // 2
H = W = 1024
IMGS_PER_CORE = 2
N_CORES = 8

SLAB = 128
NSLAB = H // SLAB  # 8
GRP = 4            # slabs per input DMA batch
NGRP = NSLAB // GRP

OUT_BIAS = 0.0           # output stored as plain fp16
PB_PAD_VAL = 0.0

# scan geometry: window sum S[v] = sum_{u in (v-15, v]} t1[u], t1 padded
# with PB_PAD_VAL.  padbuf cols [PB_LO, PB_LO+W) hold t1; data0 =
# [PB_LO:PB_LO+NV), data1 = [0:NV), out col v maps to image col v-PAD.
PB_LO = KS          # 15 pad cols in front (data1 reads t1[v-15])
NV = W + PAD        # scan positions
PB_W = KS + W + PAD  # 1046

LAST_RESULTS = None
_PROGRAM_CACHE = {}


def _build_bmat():
    """Band matrices as one [128, 384] fp16 tensor (k = partition).

    cols   0:128  B_main: B[k, m] = 1 iff |k-m| <= 7    (this slab's rows)
    cols 128:256  B_prev: B[k, m] = 1 iff 121 <= k-m <= 127 (prev slab's
                  rows read as a base-64 64-deep contraction)
    cols 256:384  B_next: B[k, m] = 1 iff 121+k <= m     (next slab's first 7
                  rows), k in [0, 7)
    """
    B = np.zeros((128, 3 * SLAB), dtype=np.float32)
    k = np.arange(128)[:, None]
    m = np.arange(SLAB)[None, :]
    B[:, 0:SLAB] = (np.abs(k - m) <= PAD).astype(np.float32)
    B[:, SLAB : 2 * SLAB] = ((k - m >= 121) & (k - m <= 127)).astype(np.float32)
    B[:, 2 * SLAB : 3 * SLAB] = ((k <= 2 * PAD - 1) & (m >= 121 + k)).astype(
        np.float32
    )
    return B.astype(np.float16)


class _Slabs:
    """Per-(image, slab) emission helpers so stages can be staggered."""

    def __init__(self, nc, x, y, bmat, scale, darks, padbufs, pools, mode,
                 bias_ap=None):
        self.nc = nc
        self.x = x
        self.y = y
        self.bmat = bmat
        self.scale = scale
        self.darks = darks
        self.padbufs = padbufs
        self.chpool, self.c2pool, self.mpool, self.opool, self.pspool = pools
        self.mode = mode
        self.bias_ap = bias_ap
        self.groups = {}
        self.psums = {}

    def load_group(self, i, t):
        """One casting SWDGE DMA for channels 0,1 (fp8->fp16) and one raw
        fp8 HWDGE DMA for channel 2, per slab."""
        nc = self.nc
        r0 = t * SLAB
        ch = self.chpool.tile([128, 2 * W], F16, tag="ch")
        nc.gpsimd.dma_start(
            ch[:].rearrange("p (c w) -> p c w", c=2),
            self.x[i, 0:2, r0 : r0 + SLAB, :].rearrange("c p w -> p c w"),
        )
        c2 = self.c2pool.tile([128, W], F8, tag="c2")
        nc.sync.dma_start(c2[:], self.x[i, 2, r0 : r0 + SLAB, :])
        self.groups[(i, t)] = (ch, c2)
        if self.mode == "dma":
            nc.sync.dma_start(self.y[i, r0 : r0 + SLAB, :], ch[0:SLAB, 0:W])

    def minim(self, i, t):
        nc = self.nc
        ch, c2 = self.groups.pop((i, t))
        # ScalarE upcast of channel 2 (ScalarE has slack)
        c216 = self.mpool.tile([128, W], F16, tag="c216")
        nc.scalar.activation(
            c216[:], c2[:], mybir.ActivationFunctionType.Copy, scale=1.0,
        )
        # fp16 channel mins on DVE (2x mode)
        mt = self.mpool.tile([128, W], F16, tag="mt")
        nc.vector.tensor_tensor(
            mt[:, :], ch[:, 0:W], ch[:, W : 2 * W], mybir.AluOpType.min
        )
        nc.vector.tensor_tensor(
            self.darks[i][:, t * W : (t + 1) * W], mt[:, :], c216[:, :],
            mybir.AluOpType.min,
        )

    def matmuls(self, i, t):
        """15-tap row sums on TensorE: bands stationary, dark moving; the
        halo rows come from the neighbour slabs' dark columns via two
        extra accumulating matmuls.  Both 512-col halves land in one
        2-bank PSUM tile."""
        nc, bmat, dark = self.nc, self.bmat, self.darks[i]
        pss = []
        for half in range(2):
            c0 = half * 512
            ps = self.pspool.tile([128, 512], F32, tag="ps")
            pss.append(ps)
            mms = [
                (bmat[0:128, 0:SLAB], dark[0:128, t * W + c0 : t * W + c0 + 512])
            ]
            if t > 0 and self.mode != "mainmm":
                mms.append((
                    bmat[64:128, SLAB : 2 * SLAB],
                    dark[64:128, (t - 1) * W + c0 : (t - 1) * W + c0 + 512],
                ))
            if t < NSLAB - 1 and self.mode != "mainmm":
                mms.append((
                    bmat[0:PAD, 2 * SLAB : 3 * SLAB],
                    dark[0:PAD, (t + 1) * W + c0 : (t + 1) * W + c0 + 512],
                ))
            for k, (lhsT, rhs) in enumerate(mms):
                nc.tensor.matmul(
                    ps[:, :], lhsT=lhsT, rhs=rhs,
                    start=(k == 0), stop=(k == len(mms) - 1),
                )
        self.psums[(i, t)] = pss

    def copies(self, i, t):
        """PSUM -> padbuf (fp16) per 512-col half on ScalarE, with the
        1/225 scale and the -0.25/15 output-bias share fused."""
        nc = self.nc
        pb = self.padbufs[i * NSLAB + t]
        pss = self.psums.pop((i, t))
        for half in range(2):
            c0 = half * 512
            nc.scalar.activation(
                pb[:, PB_LO + c0 : PB_LO + c0 + 512],
                pss[half][:, :],
                mybir.ActivationFunctionType.Copy,
                scale=self.scale,
            )

    def scan_store(self, i, t):
        """15-tap column sums in one DVE scan pass, then the bias-encoded
        fp8 store on the SP ring."""
        nc = self.nc
        r0 = t * SLAB
        pb = self.padbufs[i * NSLAB + t]
        if self.mode == "noscan":
            nc.sync.dma_start(
                self.y[i, r0 : r0 + SLAB, :],
                self.opool.tile([128, W], F16, tag="out")[:, :],
            )
            return
        ot = self.opool.tile([128, NV], F16, tag="out")
        nc.vector.tensor_tensor_scan(
            ot[:, :],
            pb[:, PB_LO : PB_LO + NV],
            pb[:, 0:NV],
            0.0,
            mybir.AluOpType.add,
            mybir.AluOpType.subtract,
        )
        nc.sync.dma_start(self.y[i, r0 : r0 + SLAB, :], ot[:, PAD : PAD + W])


def _build_program(scale, reps=1, mode="full"):
    # Bacc (not raw Bass): its compile() pipeline splits multi-wait
    # instructions via event semaphores, which TRN2 walrus codegen requires.
    # reps>1 wraps the body in a For_i loop (benchmarking only).
    nc = bacc.Bacc(
        "TRN2", target_bir_lowering=False, debug=False, num_devices=N_CORES
    )
    x = nc.dram_tensor(
        "x", [IMGS_PER_CORE, 3, H, W], F8, kind="ExternalInput"
    )
    bm = nc.dram_tensor("bmat", [128, 3 * SLAB], F16, kind="ExternalInput")
    y = nc.dram_tensor("y", [IMGS_PER_CORE, H, W], F16, kind="ExternalOutput")

    with tile.TileContext(nc) as tc:
        with (
            tc.tile_pool(name="const", bufs=1) as cpool,
            tc.tile_pool(name="chan", bufs=3) as chpool,
            tc.tile_pool(name="chan2", bufs=3) as c2pool,
            tc.tile_pool(name="mintmp", bufs=3) as mpool,
            tc.tile_pool(name="darkp", bufs=1) as dpool,
            tc.tile_pool(name="pad", bufs=1) as ppool,
            tc.tile_pool(name="outp", bufs=5) as opool,
            tc.tile_pool(name="psum", bufs=8, space="PSUM") as pspool,
        ):
            bmat = cpool.tile([128, 3 * SLAB], F16)
            nc.scalar.dma_start(bmat[:], bm[:])
            bias_t = cpool.tile([128, 1], F32, name="outbias")
            nc.vector.memset(bias_t[:], PB_PAD_VAL)

            # per-image dark buffers and pad buffers live across the whole
            # program: dark so neighbouring slabs can read each other's
            # halo columns, pad so the bias-carrying edges are set once
            darks = [
                dpool.tile([128, NSLAB * W], F16, name=f"dark{i}")
                for i in range(IMGS_PER_CORE)
            ]
            padbufs = []
            for i in range(IMGS_PER_CORE):
                for t in range(NSLAB):
                    pb = ppool.tile([128, PB_W], F16, name=f"pad_{i}_{t}")
                    nc.vector.memset(pb[:, 0:PB_LO], PB_PAD_VAL)
                    nc.vector.memset(pb[:, PB_LO + W : PB_W], PB_PAD_VAL)
                    padbufs.append(pb)

            import contextlib

            loop_cm = tc.For_i(0, reps, 1) if reps > 1 else contextlib.nullcontext()
            with loop_cm:
                if mode == "noop":
                    nt = mpool.tile([128, W], F16, tag="mt")
                    nc.vector.memset(nt[:, 0:8], 0.0)
                else:
                    sl = _Slabs(nc, x, y, bmat, scale, darks, padbufs,
                                (chpool, c2pool, mpool, opool, pspool), mode,
                                bias_ap=bias_t[:, 0:1])
                    full = mode in ("full", "mainmm", "noscan")
                    compute = full or mode == "minonly"
                    # staggered schedule: load group | min t | matmuls t-1 |
                    # psum copy t-2 | scan+store t-3, then drain.  Each
                    # stage's deps finished ~2 slabs earlier, so the
                    # in-order engine queues never head-of-line block.
                    for i in range(IMGS_PER_CORE):
                        for t in range(NSLAB + 3):
                            if full and 2 <= t <= NSLAB + 1:
                                sl.copies(i, t - 2)
                            if full and t >= 3:
                                sl.scan_store(i, t - 3)
                            if t < NSLAB:
                                sl.load_group(i, t)
                            if compute and t < NSLAB:
                                sl.minim(i, t)
                            if full and 1 <= t <= NSLAB:
                                sl.matmuls(i, t - 1)
    nc.compile()
    return nc


def make_in_maps(images):
    """Per-core input dicts: fp8-cast image batch + fp16 band matrix."""
    images = np.asarray(images)
    x8 = images.astype(ml_dtypes.float8_e4m3)
    bmat = _build_bmat()
    return [
        {
            "x": x8[c * IMGS_PER_CORE : (c + 1) * IMGS_PER_CORE],
            "bmat": bmat,
        }
        for c in range(N_CORES)
    ]


def kernel(images, weight):
    global LAST_RESULTS
    images = np.ascontiguousarray(np.asarray(images, dtype=np.float32))
    weight = np.asarray(weight, dtype=np.float64)
    # reference: conv with w = weight/225; weight is uniform (ones), so the
    # whole filter reduces to mean(weight)/225 * boxsum.
    scale = float(weight.mean()) / (KS * KS)

    if scale not in _PROGRAM_CACHE:
        _PROGRAM_CACHE[scale] = _build_program(scale)
    nc = _PROGRAM_CACHE[scale]
    in_maps = make_in_maps(images)
    res = run_bass_kernel_spmd(nc, in_maps, core_ids=list(range(N_CORES)))
    LAST_RESULTS = res
    out = np.concatenate(
        [r["y"].astype(np.float32)[:, None, :, :] for r in res.results], axis=0
    )
    return out


# revision 23
# speedup vs baseline: 1.0245x; 1.0245x over previous
"""DarkChannelPrior kernel for 8 Trainium2 NeuronCores.

Computes: dark = min over 3 channels of images [16,3,1024,1024], then a
15x15 box-average convolution (padding 7) -> [16,1,1024,1024].

Strategy (streaming slab pipeline, pure data parallel, 2 images/core):
  - The host casts the f32 images to fp8 e4m3 (rel err ~2.4e-3 on the
    final output vs the 2e-2 gate) and upcasts the fp16 output back to
    f32.  HBM traffic: 6.29 MB read + 4.19 MB written per core (vs
    25.2 + 8.4 for the all-f32 pipeline, measured 106-110 us).
  - The per-core DMA-engine pool is charged for the LARGER side of each
    transfer (~330-340 GB/s), so a casting fp8->fp16 DMA costs its fp16
    destination bytes.  The input is split: channels 0,1 ride one
    gpsimd/SWDGE casting DMA per slab (fp8 HBM -> fp16 SBUF; only the
    SWDGE path can cast), channel 2 rides the SP HWDGE ring as raw fp8
    and ScalarE upcasts it per slab (ScalarE has slack).  Pool toll:
    8.39 + 2.10 in + 4.19 out = 14.7 MB ~= 45-50 us/core, the critical
    resource.
  - VectorE computes the channel min with two fp16 tensor_tensor ops
    (fp16 engages the DVE 2x_1p mode; fp8 operands would run at 1x)
    into a per-image fp16 dark buffer [128, 8*1024].
  - TensorE computes the 15-tap ROW sums with up to 3 accumulating
    banded fp16 matmuls per 512-col half (band stationary, dark moving;
    two small bands pull the conv halo from the neighbouring slabs'
    dark columns) -> ScalarE copies PSUM->padbuf (fp16) with the 1/225
    scale fused -> VectorE computes the 15-tap COLUMN sums in one
    tensor_tensor_scan (state = (t1[v] + state) - t1[v-15], fp32 state
    over zero-padded fp16 data) -> the finished fp16 rows go out on the
    SP HWDGE ring.
  - The per-slab stages are STAGGERED (load t | matmuls t-1 | psum copy
    t-2 | scan+store t-3, ready stages emitted first): every engine's
    in-order queue only sees instructions whose deps completed ~2 slabs
    ago, so nothing head-of-line blocks.

Known dead ends (measured on HW, do not revisit without new evidence):
  - fp8 elementwise on DVE runs at 1x (2x_1p needs 2-byte dtypes); the
    Pool engine has no TensorTensor at all (codegen engine check).
    The all-fp8-input design (HWDGE, no casts: input stream 23.4 us
    standalone) measures 80.2 us end-to-end vs 70.1 here: the 1x fp8
    mins make DVE (~51 us busy) the bottleneck.  The fp16 mins in this
    kernel really do get the DVE 2x mode.
  - activation() bias is silently IGNORED for Copy on this stack, and
    Identity + const-AP bias also did not apply on HW, which kills the
    "bias-encoded fp8 output" idea (plain fp8 out fails the gate at
    2.8e-2; bias-encoded would pass at 5.6e-3 if bias worked).
  - Batching input casts into 4-slab SWDGE DMAs is faster standalone
    (36.6 -> 32.2 us input stream) but SLOWER end-to-end (76.1 vs
    70.1): the coarser DMA granularity stalls the per-slab consumers.
  - Deepening the stagger from 3 to 6 slabs (with deeper pools) bought
    only ~1.9 us (72.0 -> 70.1), so chain latency is not the main
    residual; the remaining ~20 us over the 50 us stream floor is most
    consistent with TensorE p-state throttling on its bursty 6-matmul
    slabs (full-clock PE work is ~26 us/core but 0.65-1.2 GHz p-states
    put it at 41-76 us).  Next ideas: keep PE continuously busy (filler
    matmuls on always-ready operands) or restructure to 1 matmul/half
    via 114-row overlap-loaded slabs (costs +12.5% input + min work).
"""

import numpy as np
import ml_dtypes

import concourse.bacc as bacc
import concourse.bass as bass
import concourse.tile as tile
import concourse.mybir as mybir
from concourse.bass_utils import run_bass_kernel_spmd

F8 = mybir.dt.float8e4
F16 = mybir.dt.float16
F32 = mybir.dt.float32

KS = 15
PAD = KS // 2
H = W = 1024
IMGS_PER_CORE = 2
N_CORES = 8

SLAB = 128
NSLAB = H // SLAB  # 8
GRP = 4            # slabs per input DMA batch
NGRP = NSLAB // GRP

OUT_BIAS = 0.0           # output stored as plain fp16
PB_PAD_VAL = 0.0

# scan geometry: window sum S[v] = sum_{u in (v-15, v]} t1[u], t1 padded
# with PB_PAD_VAL.  padbuf cols [PB_LO, PB_LO+W) hold t1; data0 =
# [PB_LO:PB_LO+NV), data1 = [0:NV), out col v maps to image col v-PAD.
PB_LO = KS          # 15 pad cols in front (data1 reads t1[v-15])
NV = W + PAD        # scan positions
PB_W = KS + W + PAD  # 1046

LAST_RESULTS = None
_PROGRAM_CACHE = {}


def _build_bmat():
    """Band matrices as one [128, 384] fp16 tensor (k = partition).

    cols   0:128  B_main: B[k, m] = 1 iff |k-m| <= 7    (this slab's rows)
    cols 128:256  B_prev: B[k, m] = 1 iff 121 <= k-m <= 127 (prev slab's
                  rows read as a base-64 64-deep contraction)
    cols 256:384  B_next: B[k, m] = 1 iff 121+k <= m     (next slab's first 7
                  rows), k in [0, 7)
    """
    B = np.zeros((128, 3 * SLAB), dtype=np.float32)
    k = np.arange(128)[:, None]
    m = np.arange(SLAB)[None, :]
    B[:, 0:SLAB] = (np.abs(k - m) <= PAD).astype(np.float32)
    B[:, SLAB : 2 * SLAB] = ((k - m >= 121) & (k - m <= 127)).astype(np.float32)
    B[:, 2 * SLAB : 3 * SLAB] = ((k <= 2 * PAD - 1) & (m >= 121 + k)).astype(
        np.float32
    )
    return B.astype(np.float16)


class _Slabs:
    """Per-(image, slab) emission helpers so stages can be staggered."""

    def __init__(self, nc, x, y, bmat, scale, darks, padbufs, pools, mode,
                 bias_ap=None, fill_ps=None):
        self.nc = nc
        self.x = x
        self.y = y
        self.bmat = bmat
        self.scale = scale
        self.darks = darks
        self.padbufs = padbufs
        self.chpool, self.c2pool, self.mpool, self.opool, self.pspool = pools
        self.mode = mode
        self.bias_ap = bias_ap
        self.fill_ps = fill_ps
        self.groups = {}
        self.psums = {}

    def load_group(self, i, t):
        """One casting SWDGE DMA for channels 0,1 (fp8->fp16) and one raw
        fp8 HWDGE DMA for channel 2, per slab."""
        nc = self.nc
        r0 = t * SLAB
        ch = self.chpool.tile([128, 2 * W], F16, tag="ch")
        nc.gpsimd.dma_start(
            ch[:].rearrange("p (c w) -> p c w", c=2),
            self.x[i, 0:2, r0 : r0 + SLAB, :].rearrange("c p w -> p c w"),
        )
        c2 = self.c2pool.tile([128, W], F8, tag="c2")
        nc.sync.dma_start(c2[:], self.x[i, 2, r0 : r0 + SLAB, :])
        self.groups[(i, t)] = (ch, c2)
        if self.mode == "dma":
            nc.sync.dma_start(self.y[i, r0 : r0 + SLAB, :], ch[0:SLAB, 0:W])

    def minim(self, i, t):
        nc = self.nc
        ch, c2 = self.groups.pop((i, t))
        # ScalarE upcast of channel 2 (ScalarE has slack)
        c216 = self.mpool.tile([128, W], F16, tag="c216")
        nc.scalar.activation(
            c216[:], c2[:], mybir.ActivationFunctionType.Copy, scale=1.0,
        )
        # fp16 channel mins on DVE (2x mode)
        mt = self.mpool.tile([128, W], F16, tag="mt")
        nc.vector.tensor_tensor(
            mt[:, :], ch[:, 0:W], ch[:, W : 2 * W], mybir.AluOpType.min
        )
        nc.vector.tensor_tensor(
            self.darks[i][:, t * W : (t + 1) * W], mt[:, :], c216[:, :],
            mybir.AluOpType.min,
        )

    def pe_fillers(self, n):
        """Write-only matmuls on always-ready operands: keep TensorE busy
        through the gaps between slabs' real matmul bursts so the PE
        p-state ramps to (and holds) full clock."""
        nc = self.nc
        for _ in range(n):
            nc.tensor.matmul(
                self.fill_ps[:, :],
                lhsT=self.bmat[0:128, 0:SLAB],
                rhs=self.bmat[0:128, 0:SLAB],
                start=True, stop=True,
            )

    def matmuls(self, i, t):
        """15-tap row sums on TensorE: bands stationary, dark moving; the
        halo rows come from the neighbour slabs' dark columns via two
        extra accumulating matmuls.  Both 512-col halves land in one
        2-bank PSUM tile."""
        nc, bmat, dark = self.nc, self.bmat, self.darks[i]
        pss = []
        for half in range(2):
            c0 = half * 512
            ps = self.pspool.tile([128, 512], F32, tag="ps")
            pss.append(ps)
            mms = [
                (bmat[0:128, 0:SLAB], dark[0:128, t * W + c0 : t * W + c0 + 512])
            ]
            if t > 0 and self.mode != "mainmm":
                mms.append((
                    bmat[64:128, SLAB : 2 * SLAB],
                    dark[64:128, (t - 1) * W + c0 : (t - 1) * W + c0 + 512],
                ))
            if t < NSLAB - 1 and self.mode != "mainmm":
                mms.append((
                    bmat[0:PAD, 2 * SLAB : 3 * SLAB],
                    dark[0:PAD, (t + 1) * W + c0 : (t + 1) * W + c0 + 512],
                ))
            for k, (lhsT, rhs) in enumerate(mms):
                nc.tensor.matmul(
                    ps[:, :], lhsT=lhsT, rhs=rhs,
                    start=(k == 0), stop=(k == len(mms) - 1),
                )
        self.psums[(i, t)] = pss

    def copies(self, i, t):
        """PSUM -> padbuf (fp16) per 512-col half on ScalarE, with the
        1/225 scale and the -0.25/15 output-bias share fused."""
        nc = self.nc
        pb = self.padbufs[i * NSLAB + t]
        pss = self.psums.pop((i, t))
        for half in range(2):
            c0 = half * 512
            nc.scalar.activation(
                pb[:, PB_LO + c0 : PB_LO + c0 + 512],
                pss[half][:, :],
                mybir.ActivationFunctionType.Copy,
                scale=self.scale,
            )

    def scan_store(self, i, t):
        """15-tap column sums in one DVE scan pass, then the bias-encoded
        fp8 store on the SP ring."""
        nc = self.nc
        r0 = t * SLAB
        pb = self.padbufs[i * NSLAB + t]
        if self.mode == "noscan":
            nc.sync.dma_start(
                self.y[i, r0 : r0 + SLAB, :],
                self.opool.tile([128, W], F16, tag="out")[:, :],
            )
            return
        ot = self.opool.tile([128, NV], F16, tag="out")
        nc.vector.tensor_tensor_scan(
            ot[:, :],
            pb[:, PB_LO : PB_LO + NV],
            pb[:, 0:NV],
            0.0,
            mybir.AluOpType.add,
            mybir.AluOpType.subtract,
        )
        nc.sync.dma_start(self.y[i, r0 : r0 + SLAB, :], ot[:, PAD : PAD + W])


def _build_program(scale, reps=1, mode="full"):
    # Bacc (not raw Bass): its compile() pipeline splits multi-wait
    # instructions via event semaphores, which TRN2 walrus codegen requires.
    # reps>1 wraps the body in a For_i loop (benchmarking only).
    nc = bacc.Bacc(
        "TRN2", target_bir_lowering=False, debug=False, num_devices=N_CORES
    )
    x = nc.dram_tensor(
        "x", [IMGS_PER_CORE, 3, H, W], F8, kind="ExternalInput"
    )
    bm = nc.dram_tensor("bmat", [128, 3 * SLAB], F16, kind="ExternalInput")
    y = nc.dram_tensor("y", [IMGS_PER_CORE, H, W], F16, kind="ExternalOutput")

    with tile.TileContext(nc) as tc:
        with (
            tc.tile_pool(name="const", bufs=1) as cpool,
            tc.tile_pool(name="chan", bufs=6) as chpool,
            tc.tile_pool(name="chan2", bufs=6) as c2pool,
            tc.tile_pool(name="mintmp", bufs=4) as mpool,
            tc.tile_pool(name="darkp", bufs=1) as dpool,
            tc.tile_pool(name="pad", bufs=1) as ppool,
            tc.tile_pool(name="outp", bufs=7) as opool,
            tc.tile_pool(name="psum", bufs=7, space="PSUM") as pspool,
            tc.tile_pool(name="psfill", bufs=1, space="PSUM") as fpool,
        ):
            bmat = cpool.tile([128, 3 * SLAB], F16)
            nc.scalar.dma_start(bmat[:], bm[:])
            bias_t = cpool.tile([128, 1], F32, name="outbias")
            nc.vector.memset(bias_t[:], PB_PAD_VAL)
            fill_ps = fpool.tile([128, SLAB], F32, name="pefill")

            # per-image dark buffers and pad buffers live across the whole
            # program: dark so neighbouring slabs can read each other's
            # halo columns, pad so the bias-carrying edges are set once
            darks = [
                dpool.tile([128, NSLAB * W], F16, name=f"dark{i}")
                for i in range(IMGS_PER_CORE)
            ]
            padbufs = []
            for i in range(IMGS_PER_CORE):
                for t in range(NSLAB):
                    pb = ppool.tile([128, PB_W], F16, name=f"pad_{i}_{t}")
                    nc.vector.memset(pb[:, 0:PB_LO], PB_PAD_VAL)
                    nc.vector.memset(pb[:, PB_LO + W : PB_W], PB_PAD_VAL)
                    padbufs.append(pb)

            import contextlib

            loop_cm = tc.For_i(0, reps, 1) if reps > 1 else contextlib.nullcontext()
            with loop_cm:
                if mode == "noop":
                    nt = mpool.tile([128, W], F16, tag="mt")
                    nc.vector.memset(nt[:, 0:8], 0.0)
                else:
                    sl = _Slabs(nc, x, y, bmat, scale, darks, padbufs,
                                (chpool, c2pool, mpool, opool, pspool), mode,
                                bias_ap=bias_t[:, 0:1], fill_ps=fill_ps)
                    full = mode in ("full", "mainmm", "noscan")
                    compute = full or mode == "minonly"
                    # deeply staggered schedule: load+min t | matmuls t-2
                    # | psum copy t-4 | scan+store t-6, then drain.  The
                    # per-slab dependency chain carries ~14 us of engine +
                    # DMA-semaphore latency; lagging each stage by 2 slabs
                    # gives the pipeline ~6 slabs (~18 us) of slack so the
                    # latency amortizes instead of setting the slab period.
                    for i in range(IMGS_PER_CORE):
                        for t in range(NSLAB + 6):
                            if full and t <= NSLAB + 1:
                                sl.pe_fillers(6)
                            if full and 4 <= t <= NSLAB + 3:
                                sl.copies(i, t - 4)
                            if full and t >= 6:
                                sl.scan_store(i, t - 6)
                            if t < NSLAB:
                                sl.load_group(i, t)
                            if compute and t < NSLAB:
                                sl.minim(i, t)
                            if full and 2 <= t <= NSLAB + 1:
                                sl.matmuls(i, t - 2)
    nc.compile()
    return nc


def make_in_maps(images):
    """Per-core input dicts: fp8-cast image batch + fp16 band matrix."""
    images = np.asarray(images)
    x8 = images.astype(ml_dtypes.float8_e4m3)
    bmat = _build_bmat()
    return [
        {
            "x": x8[c * IMGS_PER_CORE : (c + 1) * IMGS_PER_CORE],
            "bmat": bmat,
        }
        for c in range(N_CORES)
    ]


def kernel(images, weight):
    global LAST_RESULTS
    images = np.ascontiguousarray(np.asarray(images, dtype=np.float32))
    weight = np.asarray(weight, dtype=np.float64)
    # reference: conv with w = weight/225; weight is uniform (ones), so the
    # whole filter reduces to mean(weight)/225 * boxsum.
    scale = float(weight.mean()) / (KS * KS)

    if scale not in _PROGRAM_CACHE:
        _PROGRAM_CACHE[scale] = _build_program(scale)
    nc = _PROGRAM_CACHE[scale]
    in_maps = make_in_maps(images)
    res = run_bass_kernel_spmd(nc, in_maps, core_ids=list(range(N_CORES)))
    LAST_RESULTS = res
    out = np.concatenate(
        [r["y"].astype(np.float32)[:, None, :, :] for r in res.results], axis=0
    )
    return out
